# revision 24
# baseline (speedup 1.0000x reference)
"""Trainium2 Bass kernel: multi-head self-attention block (B=16, N=1024, C=768, H=12).

Data-parallel over batch: 8 NeuronCores x 2 batches each, no collectives.

Dataflow (per core, all-transposed activations; no on-chip transposes):
  host: xT = x_shard^T                                  [C, T]
  qkT  = W_qkv[:, :2C]^T-tiles @ xT                     [2C, T]   (q^T | k^T)
  v'   = xT-tiles^T @ W_qkv[:, 2C:]  (+ ones col/head)  [T, H*(HD+1)]
  S^T  = k^T-slices^T @ q^T   (2 heads packed in PE)    [Nk, Nq]
  E    = exp(SCALE * S^T)     (ScalarE, PSUM->SBUF)
  U'   = v'^T @ E  (accum over k; row HD = softmax Z)   [HD+1, Nq]
  aoT  = U'[:HD] * (1/Z broadcast)                      [C, T]
  y    = aoT-tiles^T @ W_proj + b                       [T, C]
"""

import sys

for _p in ("/opt/trn_rl_repo", "/opt/pypackages"):
    if _p not in sys.path:
        sys.path.append(_p)

import numpy as np

B, N, C, H = 16, 1024, 768, 12
HD = C // H            # 64
SCALE = HD ** -0.5
NCORES = 8
BL = B // NCORES       # 2 batches per core
T = BL * N             # 2048 tokens per core

COMPUTE = "bf16"       # "bf16" | "f32" | "f32r"


def build_attention_nc(compute=COMPUTE, bl=BL, n=N, c=C, h=H):
    import concourse.bass as bass
    import concourse.tile as tile
    from concourse import bacc, mybir
    from contextlib import ExitStack

    hd = c // h
    t = bl * n
    scale = hd ** -0.5
    assert c % 128 == 0 and n % 512 == 0 and h % 2 == 0 and hd == 64
    CCH = c // 128      # contraction chunks over channels
    NHP = h // 2        # head pairs
    NQ = n // 512       # 512-wide q tiles per sequence
    NKT = n // 128      # 128-wide k tiles per sequence
    NTT = n // 128      # 128-wide token tiles per sequence
    VW = hd + 1         # v' width per head (ones col at hd)
    PH = c // 2         # proj/v free-dim half (768/2=384), <= 512 & <= 1 PSUM bank
    assert PH <= 512

    FP32 = mybir.dt.float32
    SD = mybir.dt.bfloat16 if compute == "bf16" else FP32  # storage dtype

    def mm(ap):
        # matmul-operand view; f32r = fast single-pass fp32 path on TRN2 PE
        return ap.bitcast(mybir.dt.float32r) if compute == "f32r" else ap

    nc = bacc.Bacc("TRN2", target_bir_lowering=False, debug=False,
                   num_devices=NCORES)

    # inputs arrive pre-cast to the storage dtype (host-side cast)
    xT_d = nc.dram_tensor("xT", [c, t], SD, kind="ExternalInput").ap()
    wqkv_d = nc.dram_tensor("w_qkv", [c, 3 * c], SD, kind="ExternalInput").ap()
    wproj_d = nc.dram_tensor("w_proj", [c, c], SD, kind="ExternalInput").ap()
    bias_d = nc.dram_tensor("bias", [128, c], FP32, kind="ExternalInput").ap()
    out_d = nc.dram_tensor("out", [t, c], FP32, kind="ExternalOutput").ap()

    Exp = mybir.ActivationFunctionType.Exp

    with tile.TileContext(nc) as tc, ExitStack() as ctx:
        consts = ctx.enter_context(tc.tile_pool(name="consts", bufs=1))
        big = 2 if SD != FP32 else 1     # cross-batch double buffering
        xp = ctx.enter_context(tc.tile_pool(name="xp", bufs=2))
        qkp = ctx.enter_context(tc.tile_pool(name="qkp", bufs=3))
        vp = ctx.enter_context(tc.tile_pool(name="vp", bufs=big))
        ep = ctx.enter_context(tc.tile_pool(name="ep", bufs=3))
        aop = ctx.enter_context(tc.tile_pool(name="aop", bufs=big))
        smp = ctx.enter_context(tc.tile_pool(name="smp", bufs=2))
        yp = ctx.enter_context(tc.tile_pool(name="yp", bufs=3))
        ps_s = ctx.enter_context(tc.tile_pool(name="ps_s", bufs=2, space="PSUM"))
        ps_u = ctx.enter_context(tc.tile_pool(name="ps_u", bufs=4, space="PSUM"))

        # --- weights + batch-0 x, interleaved per chunk so compute can start
        # as soon as chunk 0 of each has landed ---
        wqkv_sb = []
        wproj_sb = []
        xT_b0 = []
        for cc in range(CCH):
            xt = xp.tile([128, n], SD, tag=f"x{cc}", name=f"x_b0c{cc}")
            nc.sync.dma_start(out=xt, in_=xT_d[cc * 128:(cc + 1) * 128, 0:n])
            xT_b0.append(xt)
            w1 = consts.tile([128, 3 * c], SD, tag=f"wqkv{cc}")
            nc.sync.dma_start(out=w1, in_=wqkv_d[cc * 128:(cc + 1) * 128, :])
            wqkv_sb.append(w1)
        for cc in range(CCH):
            w2 = consts.tile([128, c], SD, tag=f"wproj{cc}")
            nc.sync.dma_start(out=w2, in_=wproj_d[cc * 128:(cc + 1) * 128, :])
            wproj_sb.append(w2)
        bias_sb = consts.tile([128, c], FP32, tag="bias")
        nc.sync.dma_start(out=bias_sb, in_=bias_d)

        # --- remaining batches' x ---
        xT_all = [xT_b0]
        for b in range(1, bl):
            xT_sb = []
            for cc in range(CCH):
                xt = xp.tile([128, n], SD, tag=f"x{cc}", name=f"x_b{b}c{cc}")
                src = xT_d[cc * 128:(cc + 1) * 128, b * n:(b + 1) * n]
                nc.sync.dma_start(out=xt, in_=src)
                xT_sb.append(xt)
            xT_all.append(xT_sb)

        # --- v' tiles per batch: [128 tok, h*VW], ones col per head at hd ---
        v_all = []
        for b in range(bl):
            v_sb = []
            for tt in range(NTT):
                vt = vp.tile([128, h * VW], SD, tag=f"v{tt}", name=f"v_b{b}t{tt}")
                ones_view = vt[:, :].rearrange("p (hh w) -> p hh w", hh=h)[:, :, hd:hd + 1]
                nc.gpsimd.memset(ones_view, 1.0)
                for half in range(2):
                    ps = ps_s.tile([128, PH], FP32, tag="s", name=f"vps_b{b}t{tt}f{half}")
                    for cc in range(CCH):
                        nc.tensor.matmul(
                            ps,
                            lhsT=mm(xT_all[b][cc][:, tt * 128:(tt + 1) * 128]),
                            rhs=mm(wqkv_sb[cc][:, 2 * c + half * PH:
                                               2 * c + (half + 1) * PH]),
                            start=(cc == 0), stop=(cc == CCH - 1))
                    # strided copy into per-head 64-wide slices (skip ones col)
                    nheads = PH // hd
                    dst = vt[:, half * nheads * VW:(half + 1) * nheads * VW].rearrange(
                        "p (hh w) -> p hh w", hh=nheads)[:, :, 0:hd]
                    srcv = ps[:].rearrange("p (hh w) -> p hh w", hh=nheads)
                    with tc.high_priority(offset=300):
                        nc.vector.tensor_copy(dst, srcv)
                v_sb.append(vt)
            v_all.append(v_sb)

        # --- per head pair: q^T/k^T projection (all batches), then attention
        # for each batch ---
        aoT_all = [[] for _ in range(bl)]
        tn = bl * n
        for hp in range(NHP):
            # q^T pair tile (2 heads stacked) and k^T pair tile, all batches
            # wide: the stationary W slice is reused across bl*n/512 matmuls
            qt = qkp.tile([128, tn], SD, tag="qt", name=f"qt{hp}")
            kt_ = qkp.tile([128, tn], SD, tag="kt", name=f"kt{hp}")
            for dst, fbase in ((qt, hp * 128), (kt_, c + hp * 128)):
                for qn in range(tn // 512):
                    b_of = qn // (n // 512)
                    qq = qn % (n // 512)
                    ps = ps_s.tile([128, 512], FP32, tag="s",
                                   name=f"qkps{hp}_{qn}")
                    for cc in range(CCH):
                        nc.tensor.matmul(
                            ps,
                            lhsT=mm(wqkv_sb[cc][:, fbase:fbase + 128]),
                            rhs=mm(xT_all[b_of][cc][:, qq * 512:(qq + 1) * 512]),
                            start=(cc == 0), stop=(cc == CCH - 1))
                    with tc.high_priority(offset=300):
                        nc.vector.tensor_copy(dst[:, qn * 512:(qn + 1) * 512], ps)

            for b in range(bl):
                qb = qt[:, b * n:(b + 1) * n]
                kb = kt_[:, b * n:(b + 1) * n]
                u_ps = [[ps_u.tile([VW, 512], FP32, tag="u",
                                   name=f"u_b{b}hp{hp}h{hh}q{qn}")
                         for qn in range(NQ)]
                        for hh in range(2)]
                for kt in range(NKT):
                    # S matmuls for both heads back-to-back: alternating PE
                    # row-halves lets LDWEIGHTS prefetch ahead
                    sps_l = []
                    for head in range(2):
                        p0 = head * 64
                        sps = ps_s.tile([128, n], FP32, tag="s",
                                        name=f"s_b{b}hp{hp}k{kt}h{head}")
                        for qn in range(NQ):
                            nc.tensor.matmul(
                                sps[:, qn * 512:(qn + 1) * 512],
                                lhsT=mm(kb[p0:p0 + 64, kt * 128:(kt + 1) * 128]),
                                rhs=mm(qb[p0:p0 + 64, qn * 512:(qn + 1) * 512]),
                                start=True, stop=True)
                        sps_l.append(sps)
                    ets = []
                    for head in range(2):
                        et = ep.tile([128, n], SD, tag=f"e{head}",
                                     name=f"e_b{b}hp{hp}k{kt}h{head}")
                        nc.scalar.activation(et, sps_l[head], Exp, scale=scale)
                        ets.append(et)
                    for head in range(2):
                        hh = 2 * hp + head
                        for qn in range(NQ):
                            nc.tensor.matmul(
                                u_ps[head][qn],
                                lhsT=mm(v_all[b][kt][:, hh * VW:hh * VW + VW]),
                                rhs=mm(ets[head][:, qn * 512:(qn + 1) * 512]),
                                start=(kt == 0), stop=(kt == NKT - 1))

                # normalize: aoT[hp] rows 0:64 = head A, 64:128 = head B.
                # U-PSUM evacuates to SBUF first so the banks free up fast;
                # the rest of the chain runs off the PE critical path.
                ao = aop.tile([128, n], SD, tag=f"ao{hp}", name=f"ao_b{b}hp{hp}")
                for head in range(2):
                    usb = smp.tile([VW, n], FP32, tag=f"usb{head}",
                                   name=f"usb_b{b}hp{hp}h{head}")
                    for qn in range(NQ):
                        nc.vector.tensor_copy(usb[:, qn * 512:(qn + 1) * 512],
                                              u_ps[head][qn])
                    # Z row -> partition 0 (DMA), broadcast to 64 partitions
                    # (gpsimd), then reciprocal on the full-width tile (the
                    # custom DVE op mis-executes on 1-partition slices at
                    # base partition != 0).
                    z1 = smp.tile([1, n], FP32, tag=f"z1{head}", bufs=1,
                                  name=f"z1_b{b}hp{hp}h{head}")
                    nc.gpsimd.dma_start(out=z1, in_=usb[hd:hd + 1, :])
                    rb = smp.tile([64, n], FP32, tag=f"rb{head}",
                                  name=f"rb_b{b}hp{hp}h{head}")
                    nc.gpsimd.partition_broadcast(rb, z1)
                    nc.vector.reciprocal_approx_fast(rb, rb)
                    if head == 0:
                        nc.gpsimd.tensor_mul(ao[0:64, :], usb[0:hd, :], rb)
                    else:
                        sc = smp.tile([64, n], SD, tag="sc",
                                      name=f"sc_b{b}hp{hp}")
                        nc.gpsimd.tensor_mul(sc, usb[0:hd, :], rb)
                        nc.gpsimd.dma_start(out=ao[64:128, :], in_=sc)
                aoT_all[b].append(ao)

        # --- output projection + bias ---
        for b in range(bl):
            for tt in range(NTT):
                for half in range(2):
                    ps = ps_s.tile([128, PH], FP32, tag="s",
                                   name=f"yps_b{b}t{tt}f{half}")
                    for cc in range(CCH):
                        nc.tensor.matmul(
                            ps,
                            lhsT=mm(aoT_all[b][cc][:, tt * 128:(tt + 1) * 128]),
                            rhs=mm(wproj_sb[cc][:, half * PH:(half + 1) * PH]),
                            start=(cc == 0), stop=(cc == CCH - 1))
                    yt = yp.tile([128, PH], FP32, tag="y", name=f"y_b{b}t{tt}f{half}")
                    nc.vector.tensor_add(yt, ps,
                                         bias_sb[:, half * PH:(half + 1) * PH])
                    nc.sync.dma_start(
                        out=out_d[b * n + tt * 128:b * n + (tt + 1) * 128,
                                  half * PH:(half + 1) * PH],
                        in_=yt)

    nc.compile()
    return nc


_NC_CACHE = {}


def _get_nc(compute=COMPUTE):
    if compute not in _NC_CACHE:
        _NC_CACHE[compute] = build_attention_nc(compute)
    return _NC_CACHE[compute]


def make_in_maps(x, W_qkv, W_proj, b_proj, compute=None):
    compute = compute or COMPUTE
    if compute == "bf16":
        import ml_dtypes
        sd = ml_dtypes.bfloat16
    else:
        sd = np.float32
    x = np.asarray(x, dtype=np.float32)
    W_qkv = np.ascontiguousarray(np.asarray(W_qkv, dtype=np.float32)).astype(sd)
    W_proj = np.ascontiguousarray(np.asarray(W_proj, dtype=np.float32)).astype(sd)
    bias = np.ascontiguousarray(
        np.broadcast_to(np.asarray(b_proj, dtype=np.float32), (128, C)))
    in_maps = []
    for i in range(NCORES):
        shard = x[i * BL:(i + 1) * BL]                      # [BL, N, C]
        xT = np.ascontiguousarray(shard.transpose(2, 0, 1).reshape(C, T)).astype(sd)
        in_maps.append({"xT": xT, "w_qkv": W_qkv, "w_proj": W_proj,
                        "bias": bias})
    return in_maps


def kernel(x, W_qkv, W_proj, b_proj):
    from concourse.bass_utils import run_bass_kernel_spmd

    nc = _get_nc()
    in_maps = make_in_maps(x, W_qkv, W_proj, b_proj)
    res = run_bass_kernel_spmd(nc, in_maps, core_ids=list(range(NCORES)))
    outs = [res.results[i]["out"].reshape(BL, N, C) for i in range(NCORES)]
    return np.concatenate(outs, axis=0).astype(np.float32)


if __name__ == "__main__":
    nc = build_attention_nc()
    print("built ok")


# revision 25
# speedup vs baseline: 1.1867x; 1.1867x over previous
"""Trainium2 Bass kernel: multi-head self-attention block (B=16, N=1024, C=768, H=12).

Data-parallel over batch: 8 NeuronCores x 2 batches each, no collectives.

Dataflow (per core, all-transposed activations; no on-chip transposes):
  host: xT = x_shard^T                                  [C, T]
  qkT  = W_qkv[:, :2C]^T-tiles @ xT                     [2C, T]   (q^T | k^T)
  v'   = xT-tiles^T @ W_qkv[:, 2C:]  (+ ones col/head)  [T, H*(HD+1)]
  S^T  = k^T-slices^T @ q^T   (2 heads packed in PE)    [Nk, Nq]
  E    = exp(SCALE * S^T)     (ScalarE, PSUM->SBUF)
  U'   = v'^T @ E  (accum over k; row HD = softmax Z)   [HD+1, Nq]
  aoT  = U'[:HD] * (1/Z broadcast)                      [C, T]
  y    = aoT-tiles^T @ W_proj + b                       [T, C]
"""

import sys

for _p in ("/opt/trn_rl_repo", "/opt/pypackages"):
    if _p not in sys.path:
        sys.path.append(_p)

import numpy as np

B, N, C, H = 16, 1024, 768, 12
HD = C // H            # 64
SCALE = HD ** -0.5
NCORES = 8
BL = B // NCORES       # 2 batches per core
T = BL * N             # 2048 tokens per core

COMPUTE = "bf16"       # "bf16" | "f32" | "f32r"


def build_attention_nc(compute=COMPUTE, bl=BL, n=N, c=C, h=H):
    import concourse.bass as bass
    import concourse.tile as tile
    from concourse import bacc, mybir
    from contextlib import ExitStack

    hd = c // h
    t = bl * n
    scale = hd ** -0.5
    assert c % 128 == 0 and n % 512 == 0 and h % 2 == 0 and hd == 64
    CCH = c // 128      # contraction chunks over channels
    NHP = h // 2        # head pairs
    NQ = n // 512       # 512-wide q tiles per sequence
    NKT = n // 128      # 128-wide k tiles per sequence
    NTT = n // 128      # 128-wide token tiles per sequence
    VW = hd + 1         # v' width per head (ones col at hd)
    PH = c // 2         # proj/v free-dim half (768/2=384), <= 512 & <= 1 PSUM bank
    assert PH <= 512

    FP32 = mybir.dt.float32
    SD = mybir.dt.bfloat16 if compute == "bf16" else FP32  # storage dtype

    def mm(ap):
        # matmul-operand view; f32r = fast single-pass fp32 path on TRN2 PE
        return ap.bitcast(mybir.dt.float32r) if compute == "f32r" else ap

    nc = bacc.Bacc("TRN2", target_bir_lowering=False, debug=False,
                   num_devices=NCORES)

    # inputs arrive pre-cast to the storage dtype (host-side cast)
    xT_d = nc.dram_tensor("xT", [c, t], SD, kind="ExternalInput").ap()
    wqkv_d = nc.dram_tensor("w_qkv", [c, 3 * c], SD, kind="ExternalInput").ap()
    wproj_d = nc.dram_tensor("w_proj", [c, c], SD, kind="ExternalInput").ap()
    bias_d = nc.dram_tensor("bias", [128, c], FP32, kind="ExternalInput").ap()
    out_d = nc.dram_tensor("out", [t, c], FP32, kind="ExternalOutput").ap()

    Exp = mybir.ActivationFunctionType.Exp

    with tile.TileContext(nc) as tc, ExitStack() as ctx:
        consts = ctx.enter_context(tc.tile_pool(name="consts", bufs=1))
        big = 2 if SD != FP32 else 1     # cross-batch double buffering
        xp = ctx.enter_context(tc.tile_pool(name="xp", bufs=2))
        qkp = ctx.enter_context(tc.tile_pool(name="qkp", bufs=3))
        vp = ctx.enter_context(tc.tile_pool(name="vp", bufs=big))
        ep = ctx.enter_context(tc.tile_pool(name="ep", bufs=3))
        aop = ctx.enter_context(tc.tile_pool(name="aop", bufs=big))
        smp = ctx.enter_context(tc.tile_pool(name="smp", bufs=2))
        yp = ctx.enter_context(tc.tile_pool(name="yp", bufs=3))
        ps_s = ctx.enter_context(tc.tile_pool(name="ps_s", bufs=2, space="PSUM"))
        ps_u = ctx.enter_context(tc.tile_pool(name="ps_u", bufs=4, space="PSUM"))

        # --- weights + batch-0 x, interleaved per chunk so compute can start
        # as soon as chunk 0 of each has landed ---
        wqkv_sb = []
        wproj_sb = []
        xT_b0 = []
        for cc in range(CCH):
            xt = xp.tile([128, n], SD, tag=f"x{cc}", name=f"x_b0c{cc}")
            nc.sync.dma_start(out=xt, in_=xT_d[cc * 128:(cc + 1) * 128, 0:n])
            xT_b0.append(xt)
            w1 = consts.tile([128, 3 * c], SD, tag=f"wqkv{cc}")
            nc.sync.dma_start(out=w1, in_=wqkv_d[cc * 128:(cc + 1) * 128, :])
            wqkv_sb.append(w1)
        for cc in range(CCH):
            w2 = consts.tile([128, c], SD, tag=f"wproj{cc}")
            nc.sync.dma_start(out=w2, in_=wproj_d[cc * 128:(cc + 1) * 128, :])
            wproj_sb.append(w2)
        bias_sb = consts.tile([128, c], FP32, tag="bias")
        nc.sync.dma_start(out=bias_sb, in_=bias_d)

        # --- remaining batches' x ---
        xT_all = [xT_b0]
        for b in range(1, bl):
            xT_sb = []
            for cc in range(CCH):
                xt = xp.tile([128, n], SD, tag=f"x{cc}", name=f"x_b{b}c{cc}")
                src = xT_d[cc * 128:(cc + 1) * 128, b * n:(b + 1) * n]
                nc.sync.dma_start(out=xt, in_=src)
                xT_sb.append(xt)
            xT_all.append(xT_sb)

        # --- v' tiles per batch: [128 tok, h*VW], ones col per head at hd ---
        v_all = []
        for b in range(bl):
            v_sb = []
            for tt in range(NTT):
                vt = vp.tile([128, h * VW], SD, tag=f"v{tt}", name=f"v_b{b}t{tt}")
                ones_view = vt[:, :].rearrange("p (hh w) -> p hh w", hh=h)[:, :, hd:hd + 1]
                nc.gpsimd.memset(ones_view, 1.0)
                for half in range(2):
                    ps = ps_s.tile([128, PH], FP32, tag="s", name=f"vps_b{b}t{tt}f{half}")
                    for cc in range(CCH):
                        nc.tensor.matmul(
                            ps,
                            lhsT=mm(xT_all[b][cc][:, tt * 128:(tt + 1) * 128]),
                            rhs=mm(wqkv_sb[cc][:, 2 * c + half * PH:
                                               2 * c + (half + 1) * PH]),
                            start=(cc == 0), stop=(cc == CCH - 1))
                    # strided copy into per-head 64-wide slices (skip ones col)
                    nheads = PH // hd
                    dst = vt[:, half * nheads * VW:(half + 1) * nheads * VW].rearrange(
                        "p (hh w) -> p hh w", hh=nheads)[:, :, 0:hd]
                    srcv = ps[:].rearrange("p (hh w) -> p hh w", hh=nheads)
                    with tc.high_priority(offset=300):
                        nc.vector.tensor_copy(dst, srcv)
                v_sb.append(vt)
            v_all.append(v_sb)

        # --- per head pair: q^T/k^T projection (all batches), then attention
        # for each batch ---
        aoT_all = [[] for _ in range(bl)]
        tn = bl * n
        for hp in range(NHP):
            # q^T pair tile (2 heads stacked) and k^T pair tile, all batches
            # wide: the stationary W slice is reused across bl*n/512 matmuls
            qt = qkp.tile([128, tn], SD, tag="qt", name=f"qt{hp}")
            kt_ = qkp.tile([128, tn], SD, tag="kt", name=f"kt{hp}")
            for dst, fbase in ((qt, hp * 128), (kt_, c + hp * 128)):
                for qn in range(tn // 512):
                    b_of = qn // (n // 512)
                    qq = qn % (n // 512)
                    ps = ps_s.tile([128, 512], FP32, tag="s",
                                   name=f"qkps{hp}_{qn}")
                    for cc in range(CCH):
                        nc.tensor.matmul(
                            ps,
                            lhsT=mm(wqkv_sb[cc][:, fbase:fbase + 128]),
                            rhs=mm(xT_all[b_of][cc][:, qq * 512:(qq + 1) * 512]),
                            start=(cc == 0), stop=(cc == CCH - 1))
                    with tc.high_priority(offset=300):
                        nc.vector.tensor_copy(dst[:, qn * 512:(qn + 1) * 512], ps)

            for b in range(bl):
                qb = qt[:, b * n:(b + 1) * n]
                kb = kt_[:, b * n:(b + 1) * n]
                u_ps = [[ps_u.tile([VW, 512], FP32, tag="u",
                                   name=f"u_b{b}hp{hp}h{hh}q{qn}")
                         for qn in range(NQ)]
                        for hh in range(2)]
                for kt in range(NKT):
                    # S matmuls for both heads back-to-back: alternating PE
                    # row-halves lets LDWEIGHTS prefetch ahead
                    sps_l = []
                    for head in range(2):
                        p0 = head * 64
                        sps = ps_s.tile([128, n], FP32, tag="s",
                                        name=f"s_b{b}hp{hp}k{kt}h{head}")
                        for qn in range(NQ):
                            nc.tensor.matmul(
                                sps[:, qn * 512:(qn + 1) * 512],
                                lhsT=mm(kb[p0:p0 + 64, kt * 128:(kt + 1) * 128]),
                                rhs=mm(qb[p0:p0 + 64, qn * 512:(qn + 1) * 512]),
                                start=True, stop=True)
                        sps_l.append(sps)
                    ets = []
                    for head in range(2):
                        et = ep.tile([128, n], SD, tag=f"e{head}",
                                     name=f"e_b{b}hp{hp}k{kt}h{head}")
                        nc.scalar.activation(et, sps_l[head], Exp, scale=scale)
                        ets.append(et)
                    for head in range(2):
                        hh = 2 * hp + head
                        for qn in range(NQ):
                            nc.tensor.matmul(
                                u_ps[head][qn],
                                lhsT=mm(v_all[b][kt][:, hh * VW:hh * VW + VW]),
                                rhs=mm(ets[head][:, qn * 512:(qn + 1) * 512]),
                                start=(kt == 0), stop=(kt == NKT - 1))

                # normalize: aoT[hp] rows 0:64 = head A, 64:128 = head B.
                # U-PSUM evacuates to SBUF first so the banks free up fast;
                # the rest of the chain runs off the PE critical path.
                ao = aop.tile([128, n], SD, tag=f"ao{hp}", name=f"ao_b{b}hp{hp}")
                for head in range(2):
                    usb = smp.tile([VW, n], FP32, tag=f"usb{head}",
                                   name=f"usb_b{b}hp{hp}h{head}")
                    for qn in range(NQ):
                        nc.vector.tensor_copy(usb[:, qn * 512:(qn + 1) * 512],
                                              u_ps[head][qn])
                    # Z row -> partition 0 (DMA), broadcast to 64 partitions
                    # (gpsimd), then reciprocal on the full-width tile (the
                    # custom DVE op mis-executes on 1-partition slices at
                    # base partition != 0).
                    z1 = smp.tile([1, n], FP32, tag=f"z1{head}", bufs=1,
                                  name=f"z1_b{b}hp{hp}h{head}")
                    nc.gpsimd.dma_start(out=z1, in_=usb[hd:hd + 1, :])
                    rb = smp.tile([64, n], FP32, tag=f"rb{head}",
                                  name=f"rb_b{b}hp{hp}h{head}")
                    nc.gpsimd.partition_broadcast(rb, z1)
                    nc.vector.reciprocal_approx_fast(rb, rb)
                    if head == 0:
                        nc.vector.tensor_mul(ao[0:64, :], usb[0:hd, :], rb)
                    else:
                        sc = smp.tile([64, n], SD, tag="sc",
                                      name=f"sc_b{b}hp{hp}")
                        nc.vector.tensor_mul(sc, usb[0:hd, :], rb)
                        nc.gpsimd.dma_start(out=ao[64:128, :], in_=sc)
                aoT_all[b].append(ao)

        # --- output projection + bias ---
        for b in range(bl):
            for tt in range(NTT):
                for half in range(2):
                    ps = ps_s.tile([128, PH], FP32, tag="s",
                                   name=f"yps_b{b}t{tt}f{half}")
                    for cc in range(CCH):
                        nc.tensor.matmul(
                            ps,
                            lhsT=mm(aoT_all[b][cc][:, tt * 128:(tt + 1) * 128]),
                            rhs=mm(wproj_sb[cc][:, half * PH:(half + 1) * PH]),
                            start=(cc == 0), stop=(cc == CCH - 1))
                    yt = yp.tile([128, PH], FP32, tag="y", name=f"y_b{b}t{tt}f{half}")
                    nc.vector.tensor_add(yt, ps,
                                         bias_sb[:, half * PH:(half + 1) * PH])
                    nc.sync.dma_start(
                        out=out_d[b * n + tt * 128:b * n + (tt + 1) * 128,
                                  half * PH:(half + 1) * PH],
                        in_=yt)

    nc.compile()
    return nc


_NC_CACHE = {}


def _get_nc(compute=COMPUTE):
    if compute not in _NC_CACHE:
        _NC_CACHE[compute] = build_attention_nc(compute)
    return _NC_CACHE[compute]


def make_in_maps(x, W_qkv, W_proj, b_proj, compute=None):
    compute = compute or COMPUTE
    if compute == "bf16":
        import ml_dtypes
        sd = ml_dtypes.bfloat16
    else:
        sd = np.float32
    x = np.asarray(x, dtype=np.float32)
    W_qkv = np.ascontiguousarray(np.asarray(W_qkv, dtype=np.float32)).astype(sd)
    W_proj = np.ascontiguousarray(np.asarray(W_proj, dtype=np.float32)).astype(sd)
    bias = np.ascontiguousarray(
        np.broadcast_to(np.asarray(b_proj, dtype=np.float32), (128, C)))
    in_maps = []
    for i in range(NCORES):
        shard = x[i * BL:(i + 1) * BL]                      # [BL, N, C]
        xT = np.ascontiguousarray(shard.transpose(2, 0, 1).reshape(C, T)).astype(sd)
        in_maps.append({"xT": xT, "w_qkv": W_qkv, "w_proj": W_proj,
                        "bias": bias})
    return in_maps


def kernel(x, W_qkv, W_proj, b_proj):
    from concourse.bass_utils import run_bass_kernel_spmd

    nc = _get_nc()
    in_maps = make_in_maps(x, W_qkv, W_proj, b_proj)
    res = run_bass_kernel_spmd(nc, in_maps, core_ids=list(range(NCORES)))
    outs = [res.results[i]["out"].reshape(BL, N, C) for i in range(NCORES)]
    return np.concatenate(outs, axis=0).astype(np.float32)


if __name__ == "__main__":
    nc = build_attention_nc()
    print("built ok")
